# revision 1
# baseline (speedup 1.0000x reference)
"""Multi-head attention (b=2, n=2048, e=1024, h=16, d=64) on 8 trn2 NeuronCores.

Sharding: data-parallel over batch (2) x tensor-parallel over heads (16 -> 4
groups of 4). Core c handles batch c//4, heads 4*(c%4) .. 4*(c%4)+3.
Each core computes the qkv projection for its heads, full attention, and a
row-parallel slice of the output projection; the host sums the 4 partial
projections per batch and adds bproj.

On-chip layout is fully "transposed" (features on partitions) so softmax runs
along the free axis and no on-chip transposes are needed:
  q8,k8 [d, tok] fp8e4, with a zeroed second k-tile slot so the energy matmul
  runs in DoubleRow mode (2 rows/cycle):
      energyT[l, q] = sum_ko k8[:,ko,:].T @ q8[:,ko,:]   (ko=1 slot is zero)
  expT = exp(energyT/32)  (ScalarE, bf16 out)
  v     [tok, dcol]   (natural), per l-tile with a ones column appended
  att@v: psum[65, q] += v_ext[l,65].T @ expT[l, q]  -- row 64 accumulates the
  softmax denominator for free (ones column).
  normalize: DVE copies av psum->SBUF + reciprocal; Pool (GpSimd) broadcasts
  the reciprocal across partitions and applies it (GpSimd has no PSUM port).
  proj: out_partial[q, e] = outT.T @ Wproj_slice  (row-parallel, host reduces),
  DMA'd to DRAM straight from PSUM.

Schedule: ScalarE's exp stream (16.8M elems) is the bottleneck engine, so the
kernel is a flat software pipeline of 16 "half-units" H(i)=(head, q-chunk) in
(qc-major, head-pair) order. Each H emits its 8 energy pairs (fp8, cheap) and
the *previous* H's att@v pairs, slot by slot, so ScalarE always has a full
energy psum to exp and PE always has work while exp drains. v-projection,
the second head-pair's qk projection, and the output projection are woven
into the per-slot PE slack as explicit fillers placed just before their
consumers need the results.
"""

import numpy as np
import ml_dtypes

import concourse.bass as bass
import concourse.tile as tile
from concourse import bacc, mybir
from concourse import bass_utils

B, N, E, H, D = 2, 2048, 1024, 16, 64
NCORES = 8
HPC = H // 4  # heads per core = 4
DC = HPC * D  # dcols per core = 256
EC = E // 128  # 8 e-chunks
NT = N // 128  # 16 token tiles
QC = N // 512  # 4 q-chunks of 512
F32 = mybir.dt.float32
BF16 = mybir.dt.bfloat16
F8 = mybir.dt.float8e4
BF = ml_dtypes.bfloat16

_CACHE = {}

# energy (q.kT) matmuls in fp8e4 DoubleRow mode (2 rows/cycle) instead of bf16
FP8_ENERGY = True
# broadcast the softmax reciprocal across partitions with a K=1 PE matmul
# instead of a DRAM round-trip (measured slower on HW: the PE stalls on the
# DVE reciprocal chain at the slot-1 weave point)
BCAST_MM = False


def build_nc(debug_outs=False, reps=None, tiny_out=False):
    nc = bacc.Bacc("TRN2", target_bir_lowering=False, debug=False, num_devices=NCORES)

    xT_d = nc.dram_tensor("xT", [E, N], BF16, kind="ExternalInput")
    wq_d = nc.dram_tensor("wq", [E, DC], BF16, kind="ExternalInput")
    wk_d = nc.dram_tensor("wk", [E, DC], BF16, kind="ExternalInput")
    wv_d = nc.dram_tensor("wv", [E, DC], BF16, kind="ExternalInput")
    wp_d = nc.dram_tensor("wp", [DC, E], BF16, kind="ExternalInput")
    bqT_d = nc.dram_tensor("bqT", [DC, 1], F32, kind="ExternalInput")
    bkT_d = nc.dram_tensor("bkT", [DC, 1], F32, kind="ExternalInput")
    bvb_d = nc.dram_tensor("bvb", [128, DC], F32, kind="ExternalInput")
    out_rows = 512 if tiny_out else N
    out_d = nc.dram_tensor("out", [out_rows, E], F32, kind="ExternalOutput")

    with tile.TileContext(nc) as tc:
        with (
            tc.tile_pool(name="const", bufs=1) as const,
            tc.tile_pool(name="ps", bufs=2, space="PSUM") as ps_pool,
            tc.tile_pool(name="stg", bufs=2, space="PSUM") as stg_pool,
            tc.tile_pool(name="av", bufs=2, space="PSUM") as av_pool,
            tc.tile_pool(name="ex", bufs=12) as ex_pool,
            tc.tile_pool(name="nrm", bufs=3) as nrm_pool,
            tc.tile_pool(name="dscr", bufs=3, space="DRAM") as dscr_pool,
            tc.tile_pool(name="outst", bufs=4) as outst_pool,
        ):
            def emit_body():
                # ---- persistent SBUF tensors ----
                xT_sb = const.tile([128, EC, N], BF16)
                wq_sb = const.tile([128, EC, DC], BF16)
                wk_sb = const.tile([128, EC, DC], BF16)
                wv_sb = const.tile([128, EC, DC], BF16)
                wp_sb = const.tile([128, 2, E], BF16)
                bqT_sb = const.tile([128, 2], F32)
                bkT_sb = const.tile([128, 2], F32)
                bvb_sb = const.tile([128, DC], F32)
                # fp8 q/k: [part=(head-half, d), hc, ko, tok]; ko=1 is a zeroed
                # DoubleRow k-tile slot
                qk_dt = F8 if FP8_ENERGY else BF16
                q8 = const.tile([128, 2, 2, N], qk_dt)
                k8 = const.tile([128, 2, 2, N], qk_dt)
                # per l-tile, per head: 64 dims + ones col + pad -> lhsT [128, 66]
                v_sb = const.tile([128, NT, HPC, 66], BF16)
                outT_sb = const.tile([128, 2, N], BF16)
                ones_sb = const.tile([1, 64], BF16)

                # ---- input DMAs spread over the HWDGE queues ----
                # (gpsimd SWDGE descriptor-gen and Pool Q7 tensor ops are far
                # slower on real HW than the cost model claims -- avoid Pool)
                qeng = [nc.sync, nc.scalar]
                qi = [0]

                def dma_in(out, in_):
                    qeng[qi[0] % len(qeng)].dma_start(out=out, in_=in_)
                    qi[0] += 1

                # xT + wk first (first k-projection group waits on these),
                # then the small biases, then wq, wv, wp
                for ec in range(EC):
                    dma_in(xT_sb[:, ec, :], xT_d.ap()[ec * 128:(ec + 1) * 128, :])
                    dma_in(wk_sb[:, ec, :], wk_d.ap()[ec * 128:(ec + 1) * 128, :])
                for hc in range(2):
                    dma_in(bqT_sb[:, hc:hc + 1], bqT_d.ap()[hc * 128:(hc + 1) * 128, :])
                    dma_in(bkT_sb[:, hc:hc + 1], bkT_d.ap()[hc * 128:(hc + 1) * 128, :])
                for ec in range(EC):
                    dma_in(wq_sb[:, ec, :], wq_d.ap()[ec * 128:(ec + 1) * 128, :])
                for ec in range(EC):
                    dma_in(wv_sb[:, ec, :], wv_d.ap()[ec * 128:(ec + 1) * 128, :])
                dma_in(bvb_sb[:], bvb_d.ap())
                for hc in range(2):
                    dma_in(wp_sb[:, hc, :], wp_d.ap()[hc * 128:(hc + 1) * 128, :])

                nc.vector.memset(v_sb[:, :, :, 64:65], 1.0)
                nc.vector.memset(q8[:, :, 1, :], 0.0)
                nc.vector.memset(k8[:, :, 1, :], 0.0)
                nc.vector.memset(ones_sb[:], 1.0)

                inv_scale = 1.0 / float(np.sqrt(np.float32(E)))

                QK = ((wk_sb, bkT_sb, k8), (wq_sb, bqT_sb, q8))

                def emit_qk_group(which, m, t):
                    w_sb, b_sb, dst = QK[which]
                    pq = ps_pool.tile([128, 512], F32, tag="ps")
                    for ec in range(EC):
                        nc.tensor.matmul(
                            pq[:],
                            lhsT=w_sb[:, ec, m * 128:(m + 1) * 128],
                            rhs=xT_sb[:, ec, t * 512:(t + 1) * 512],
                            start=(ec == 0), stop=(ec == EC - 1),
                        )
                    nc.vector.tensor_scalar_add(
                        out=dst[:, m, 0, t * 512:(t + 1) * 512],
                        in0=pq[:], scalar1=b_sb[:, m:m + 1],
                    )

                def emit_v_group(lt):
                    pv = ps_pool.tile([128, DC], F32, tag="ps")
                    for ec in range(EC):
                        nc.tensor.matmul(
                            pv[:],
                            lhsT=xT_sb[:, ec, lt * 128:(lt + 1) * 128],
                            rhs=wv_sb[:, ec, :],
                            start=(ec == 0), stop=(ec == EC - 1),
                        )
                    nc.vector.tensor_add(
                        out=v_sb[:, lt, :, 0:64],
                        in0=pv[:].rearrange("p (h d) -> p h d", h=HPC),
                        in1=bvb_sb[:].rearrange("p (h d) -> p h d", h=HPC),
                    )

                # ---- attention pieces ----
                av_tiles = {}

                def emit_av1(h, qc, lt, ex, j):
                    if lt == 0:
                        av = av_pool.tile([65, 512], F32, tag="av")
                        av_tiles[(h, qc)] = av
                    av = av_tiles[(h, qc)]
                    nc.tensor.matmul(
                        av[:],
                        lhsT=v_sb[:, lt, h, 0:65],
                        rhs=ex[:, j, :],
                        start=(lt == 0), stop=(lt == NT - 1),
                    )

                def emit_slot(h, qc, r, prev=None, pex=None, self_ex=None):
                    """One pipeline slot: the E2 pair for (h,qc,r) interleaved
                    with the previous half-unit's av pair (so each DoubleRow
                    LDWEIGHTS hides under an adjacent av matmul's stream).
                    Returns the ex tile for (h, qc, r)."""
                    hc, hp = h // 2, (h % 2) * 64
                    stg = stg_pool.tile([128, 2, 512], F32, tag="stg")
                    for j in range(2):
                        lt = 2 * r + j
                        if FP8_ENERGY:
                            nc.tensor.matmul(
                                stg[:, j, :],
                                lhsT=k8[hp:hp + 64, hc, :, lt * 128:(lt + 1) * 128],
                                rhs=q8[hp:hp + 64, hc, :, qc * 512:(qc + 1) * 512],
                                start=True, stop=True,
                                perf_mode=mybir.MatmulPerfMode.DoubleRow,
                            )
                        else:
                            nc.tensor.matmul(
                                stg[:, j, :],
                                lhsT=k8[hp:hp + 64, hc, 0, lt * 128:(lt + 1) * 128],
                                rhs=q8[hp:hp + 64, hc, 0, qc * 512:(qc + 1) * 512],
                                start=True, stop=True,
                            )
                        if prev is not None:
                            emit_av1(prev[0], prev[1], 2 * r + j, pex, j)
                        if self_ex is not None:
                            emit_av1(h, qc, 2 * (r - 2) + j, self_ex, j)
                    ex = ex_pool.tile([128, 2, 512], BF16, tag="ex")
                    nc.scalar.activation(
                        out=ex[:], in_=stg[:],
                        func=mybir.ActivationFunctionType.Exp,
                        scale=inv_scale,
                    )
                    return ex

                norm_pend = {}

                def emit_norm_a(h, qc):
                    """DVE part of the softmax normalize: drain av psum, build
                    the reciprocal of the denominator row."""
                    av = av_tiles.pop((h, qc))
                    # NOTE: the DVE ISA reciprocal op needs SBUF input based at
                    # partition 0 on real HW (PSUM or partition-64 input reads
                    # garbage; CoreSim models both fine) -- stage via den_sb
                    av_sb = nrm_pool.tile([65, 512], F32, tag="avsb", bufs=4)
                    nc.vector.tensor_copy(out=av_sb[:], in_=av[:])
                    den_sb = nrm_pool.tile([1, 512], F32, tag="den")
                    nc.vector.tensor_copy(out=den_sb[:], in_=av_sb[64:65, :])
                    recip = nrm_pool.tile([1, 512], F32, tag="recip")
                    nc.vector.reciprocal_approx_fast(out=recip[:], in_=den_sb[:])
                    norm_pend[(h, qc)] = (av, av_sb, recip)

                def emit_norm_b(h, qc):
                    """Broadcast the reciprocal across 64 partitions and apply.
                    Emitted ~one half-unit after norm_a so the in-order PE
                    never waits on the DVE reciprocal chain."""
                    hc, hp = h // 2, (h % 2) * 64
                    av, av_sb, recip = norm_pend.pop((h, qc))
                    if BCAST_MM:
                        # rbc[p, q] = ones[0, p] * recip[0, q]: K=1 bf16 matmul
                        # into the (finished) av psum tile
                        recip16 = nrm_pool.tile([1, 512], BF16, tag="recip16")
                        nc.vector.tensor_copy(out=recip16[:], in_=recip[:])
                        nc.tensor.matmul(
                            av[0:64, :],
                            lhsT=ones_sb[:],
                            rhs=recip16[:],
                            start=True, stop=True,
                        )
                        rbc_ap = av[0:64, :]
                    else:
                        dscr = dscr_pool.tile([1, 512], F32, tag="dscr")
                        nc.sync.dma_start(out=dscr[:], in_=recip[:])
                        rbc = nrm_pool.tile([64, 512], F32, tag="rbc")
                        d_ap = dscr[:]
                        bcast = bass.AP(tensor=d_ap.tensor, offset=d_ap.offset,
                                        ap=[[0, 64]] + list(d_ap.ap[1:]))
                        nc.sync.dma_start(out=rbc[:], in_=bcast)
                        rbc_ap = rbc[:]
                    nc.vector.tensor_mul(
                        out=outT_sb[hp:hp + 64, hc, qc * 512:(qc + 1) * 512],
                        in0=av_sb[0:64, :],
                        in1=rbc_ap,
                    )

                # one output-projection column block (q-tile qt, 1024 wide),
                # DMA'd straight from PSUM
                def emit_proj(qt, tail=False):
                    for en in range(2):
                        po = ps_pool.tile([128, 512], F32, tag="ps")
                        for hc in range(2):
                            nc.tensor.matmul(
                                po[:],
                                lhsT=outT_sb[:, hc, qt * 128:(qt + 1) * 128],
                                rhs=wp_sb[:, hc, en * 512:(en + 1) * 512],
                                start=(hc == 0), stop=(hc == 1),
                            )
                        ot = outst_pool.tile([128, 512], F32, tag="ot")
                        # after the last exp ScalarE is idle: split the psum
                        # drain across both engines so the tail pipelines
                        if tail and en == 1:
                            nc.scalar.copy(out=ot[:], in_=po[:])
                        else:
                            nc.vector.tensor_copy(out=ot[:], in_=po[:])
                        oq = (qt % 4) if tiny_out else qt
                        (nc.scalar if (tail and en == 1) else nc.sync).dma_start(
                            out=out_d.ap()[oq * 128:(oq + 1) * 128, en * 512:(en + 1) * 512],
                            in_=ot[:],
                        )

                # ---- flat pipeline of half-units ----
                KQ = 0, 1  # which indices: 0 = k, 1 = q

                def f_qk(which, m, t):
                    return lambda: emit_qk_group(which, m, t)

                def f_v2(r2):
                    return lambda: (emit_v_group(2 * r2), emit_v_group(2 * r2 + 1))

                def f_proj(qt):
                    return lambda: emit_proj(qt)

                # H-stream: (head, qc), qc-major within each head pair
                HS = [(h0 + dh, qc) for h0 in (0, 2) for qc in range(QC)
                      for dh in (0, 1)]

                # fillers per H index, each ~<=1.7us of PE work, placed at
                # least one H before their outputs are consumed
                fillers = {
                    0: [f_v2(0), f_v2(1), f_v2(2), f_v2(3), f_v2(4)],
                    1: [f_v2(5), f_v2(6), f_v2(7), f_qk(1, 0, 1)],
                    2: [f_qk(1, 0, 2)],
                    3: [f_qk(1, 0, 3)],
                    4: [f_qk(0, 1, 0)],
                    5: [f_qk(0, 1, 1), f_qk(0, 1, 2)],
                    6: [f_qk(0, 1, 3)],
                    7: [f_qk(1, 1, 0)],
                    8: [f_qk(1, 1, 1)],
                    9: [f_qk(1, 1, 2)],
                    10: [f_qk(1, 1, 3)],
                    11: [f_proj(0), f_proj(1)],
                    12: [f_proj(2), f_proj(3)],
                    13: [f_proj(4), f_proj(5)],
                    14: [f_proj(6), f_proj(7)],
                    15: [f_proj(8), f_proj(9)],
                }
                # filler slot positions within the 8-slot H loop; proj fillers
                # must land after the slot-3 norm_b that publishes their outT
                SLOTS = (1, 2, 4, 6, 0, 3, 5, 7)
                SLOTS_LATE = (4, 6, 5, 7)

                # lead-in: k for head-pair 0 (all q-chunks' l-tiles), q chunk 0
                for t in range(QC):
                    emit_qk_group(0, 0, t)
                emit_qk_group(1, 0, 0)

                exs = {}
                last = len(HS) - 1
                for i, (h, qc) in enumerate(HS):
                    fl = list(fillers.get(i, ()))
                    fslots = {s: f for s, f in zip(SLOTS_LATE if i >= 11 else SLOTS, fl)}
                    prev = HS[i - 1] if i > 0 else None
                    for r in range(8):
                        pex = exs.pop((prev[0], prev[1], r)) if prev else None
                        # the last half-unit also drains its own avs (lag 2)
                        # so the tail chain is short
                        sex = exs[(h, qc, r - 2)] if (i == last and r >= 2) else None
                        exs[(h, qc, r)] = emit_slot(h, qc, r, prev=prev,
                                                    pex=pex, self_ex=sex)
                        if sex is not None:
                            del exs[(h, qc, r - 2)]
                        # norm_b must precede slot 2: the last half-unit's
                        # self-av pool claim needs the 2-back av buffer freed
                        if r == 1 and i >= 2:
                            emit_norm_b(*HS[i - 2])
                        if r in fslots:
                            fslots[r]()
                    if prev is not None:
                        emit_norm_a(*prev)

                # tail: final avs + norms of the last half-units, last projs
                lh, lqc = HS[-1]
                for j in range(2):
                    emit_av1(lh, lqc, 12 + j, exs[(lh, lqc, 6)], j)
                emit_proj(10, tail=True)
                for j in range(2):
                    emit_av1(lh, lqc, 14 + j, exs[(lh, lqc, 7)], j)
                emit_proj(11, tail=True)
                emit_norm_b(*HS[-2])
                emit_norm_a(lh, lqc)
                emit_norm_b(lh, lqc)
                for qt in range(12, 16):
                    emit_proj(qt, tail=True)

            if reps is None:
                emit_body()
            else:
                with tc.For_i(0, reps, 1, hint_engines=(
                        mybir.EngineType.PE, mybir.EngineType.Activation,
                        mybir.EngineType.DVE, mybir.EngineType.SP)):
                    emit_body()

    nc.compile()
    return nc


def make_in_maps(x, Wqkv, bqkv, Wproj):
    W4 = np.ascontiguousarray(Wqkv.reshape(E, H, D, 3))
    b4 = np.ascontiguousarray(bqkv.reshape(H, D, 3))
    in_maps = []
    for c in range(NCORES):
        bi, hg = c // 4, c % 4
        hs = slice(hg * HPC, (hg + 1) * HPC)
        in_maps.append({
            "xT": np.ascontiguousarray(x[bi].T).astype(BF),
            "wq": np.ascontiguousarray(W4[:, hs, :, 0].reshape(E, DC)).astype(BF),
            "wk": np.ascontiguousarray(W4[:, hs, :, 1].reshape(E, DC)).astype(BF),
            "wv": np.ascontiguousarray(W4[:, hs, :, 2].reshape(E, DC)).astype(BF),
            "wp": np.ascontiguousarray(Wproj[hg * DC:(hg + 1) * DC, :]).astype(BF),
            "bqT": np.ascontiguousarray(b4[hs, :, 0].reshape(DC, 1)).astype(np.float32),
            "bkT": np.ascontiguousarray(b4[hs, :, 1].reshape(DC, 1)).astype(np.float32),
            "bvb": np.ascontiguousarray(np.tile(b4[hs, :, 2].reshape(1, DC), (128, 1))).astype(np.float32),
        })
    return in_maps


def run(inputs, trace=False, **kw):
    if "nc" not in _CACHE:
        _CACHE["nc"] = build_nc()
    nc = _CACHE["nc"]
    in_maps = make_in_maps(inputs["x"], inputs["Wqkv"], inputs["bqkv"], inputs["Wproj"])
    res = bass_utils.run_bass_kernel_spmd(nc, in_maps, core_ids=list(range(NCORES)), trace=trace, **kw)
    out = np.zeros((B, N, E), np.float32)
    for c in range(NCORES):
        out[c // 4] += res.results[c]["out"].astype(np.float32)
    out += inputs["bproj"].astype(np.float32)[None, None, :]
    return out, res


def kernel(**inputs):
    inputs = {k: np.asarray(v) for k, v in inputs.items()}
    out, _ = run(inputs)
    return out.astype(np.float32)

